# revision 8
# baseline (speedup 1.0000x reference)
"""Trainium2 Bass kernel for LSQ weight quantizer with iterative freezing.

Pure elementwise over a [4096, 11008] weight tensor plus per-row scale.
Sharded row-wise across 8 NeuronCores (512 rows each); no communication.

Per-core layout: [512, 11008] -> 4 row-blocks of 128 partitions, free dim
tiled into chunks. All state updates are elementwise; oscillated_sum is a
full reduction finished on host (exact: summands are 0/1 ints).
"""

import os
from contextlib import ExitStack

import numpy as np

import concourse.bass as bass
import concourse.bacc as bacc
import concourse.mybir as mybir
import concourse.tile as tile
from concourse.bass_utils import run_bass_kernel_spmd

OUT_DIM, IN_DIM = 4096, 11008
N_CORES = 8
ROWS = OUT_DIM // N_CORES  # 512 rows per core
P = 128                    # SBUF partitions
RB = ROWS // P             # 4 row blocks per core
FD = 1376                  # free-dim tile size
CS = IN_DIM // FD          # 8 col steps per row block

F32 = mybir.dt.float32
U8 = mybir.dt.uint8
Alu = mybir.AluOpType
Act = mybir.ActivationFunctionType

MAGIC = float(np.float32(12582912.0))  # 1.5*2^23: (v+C)-C == round-half-even(v)
THD_NEG = -8.0
THD_POS = 7.0
MOM = float(np.float32(0.01))
ONE_MINUS_MOM = float(np.float32(0.99))
FREEZE_THR = float(np.float32(0.01))

F32_BIG = ["x", "prev_x_int", "prev_switch_dir", "ema_oscillation",
           "frozen_x_int", "ema_x_int"]
F32_OUT = ["out", "prev_x_int_new", "prev_switch_dir_new", "ema_osc_new",
           "frozen_x_int_new", "ema_x_int_new"]

_MODULE = None


def _build_module():
    nc = bacc.Bacc(None)
    d_in = {n: nc.declare_dram_parameter(n, [ROWS, IN_DIM], F32, isOutput=False)
            for n in F32_BIG}
    d_in["frozen"] = nc.declare_dram_parameter("frozen", [ROWS, IN_DIM], U8,
                                               isOutput=False)
    d_ss = nc.declare_dram_parameter("s_scale", [ROWS, 1], F32, isOutput=False)
    d_sr = nc.declare_dram_parameter("s_recip", [ROWS, 1], F32, isOutput=False)
    d_out = {n: nc.declare_dram_parameter(n, [ROWS, IN_DIM], F32, isOutput=True)
             for n in F32_OUT}
    d_out["frozen_new"] = nc.declare_dram_parameter("frozen_new", [ROWS, IN_DIM],
                                                    U8, isOutput=True)
    d_osc = nc.declare_dram_parameter("osc_rowsum", [ROWS, 1], F32, isOutput=True)

    with tile.TileContext(nc) as tc, ExitStack() as ctx:
        big = ctx.enter_context(tc.tile_pool(name="big", bufs=2))
        small = ctx.enter_context(tc.tile_pool(name="small", bufs=2))

        for rb in range(RB):
            r0 = rb * P
            ss = small.tile([P, 1], F32, tag="ss")
            nc.sync.dma_start(ss[:], d_ss[r0:r0 + P, :])
            sr = small.tile([P, 1], F32, tag="sr")
            nc.sync.dma_start(sr[:], d_sr[r0:r0 + P, :])
            acc = small.tile([P, CS], F32, tag="acc")

            for cs in range(CS):
                c0 = cs * FD

                def load(name, dt=F32, tag=None):
                    t = big.tile([P, FD], dt, tag=tag or name)
                    nc.sync.dma_start(t[:], d_in[name][r0:r0 + P, c0:c0 + FD])
                    return t

                t_x = load("x")
                t_pxi = load("prev_x_int")
                t_psd = load("prev_switch_dir")
                t_ema = load("ema_oscillation")
                t_frz = load("frozen", U8)
                t_fxi = load("frozen_x_int")
                t_exi = load("ema_x_int")

                t_xi = big.tile([P, FD], F32, tag="xi")
                t_sd = big.tile([P, FD], F32, tag="sd")
                t_delta = big.tile([P, FD], F32, tag="delta")
                t_eq = big.tile([P, FD], F32, tag="eq")
                t_prod = big.tile([P, FD], F32, tag="prod")
                t_rexi = big.tile([P, FD], F32, tag="rexi")
                t_outp = big.tile([P, FD], F32, tag="outp")
                t_mf = big.tile([P, FD], U8, tag="mf")
                t_fnew = big.tile([P, FD], U8, tag="fnew")

                # --- fake quant: x_int = clip(round(x/s), -8, 7); frozen mask ---
                # x/s lowered as x * RN(1/s), matching the XLA-neuron divide
                nc.vector.tensor_scalar(t_xi[:], t_x[:], sr[:], None, Alu.mult)
                nc.vector.tensor_scalar(t_xi[:], t_xi[:], MAGIC, MAGIC,
                                        Alu.add, Alu.subtract)
                nc.vector.tensor_scalar(t_xi[:], t_xi[:], THD_POS, THD_NEG,
                                        Alu.min, Alu.max)
                nc.vector.copy_predicated(t_xi[:], t_frz[:], t_fxi[:])

                # --- oscillation tracking ---
                nc.gpsimd.tensor_tensor(t_delta[:], t_pxi[:], t_xi[:],
                                        Alu.subtract)
                nc.scalar.sign(t_sd[:], t_delta[:])
                # eq*psd = (delta == 0) * prev_switch_dir
                nc.vector.scalar_tensor_tensor(t_eq[:], t_delta[:], 0.0,
                                               t_psd[:], Alu.is_equal, Alu.mult)
                # prod = psd * sd (before psd is dead)
                nc.vector.tensor_tensor(t_prod[:], t_psd[:], t_sd[:], Alu.mult)
                # psd_new = sd + eq*psd (in place over t_eq)
                nc.vector.tensor_tensor(t_eq[:], t_eq[:], t_sd[:], Alu.add)
                # oscillated = (prod == -1), with row-sum accumulated
                nc.vector.tensor_scalar(t_prod[:], t_prod[:], -1.0, None,
                                        Alu.is_equal, Alu.add,
                                        accum_out=acc[:, cs:cs + 1])
                # osc*0.01 on ACT (reuse t_sd)
                nc.scalar.mul(t_sd[:], t_prod[:], MOM)
                # ema_new = ema*0.99 + osc*0.01 (in place over t_ema)
                nc.vector.scalar_tensor_tensor(t_ema[:], t_ema[:],
                                               ONE_MINUS_MOM, t_sd[:],
                                               Alu.mult, Alu.add)
                # freeze_w = ema_new > 0.01 (u8 mask)
                nc.vector.tensor_scalar(t_mf[:], t_ema[:], FREEZE_THR, None,
                                        Alu.is_gt)
                # frozen_new = frozen | freeze_w
                nc.vector.tensor_tensor(t_fnew[:], t_frz[:], t_mf[:],
                                        Alu.bitwise_or)
                # round(ema_x_int)
                nc.vector.tensor_scalar(t_rexi[:], t_exi[:], MAGIC, MAGIC,
                                        Alu.add, Alu.subtract)
                # frozen_x_int_new = where(freeze_w, round(exi), fxi) in place
                nc.vector.copy_predicated(t_fxi[:], t_mf[:], t_rexi[:])
                # xi*0.01 on ACT (reuse t_delta)
                nc.scalar.mul(t_delta[:], t_xi[:], MOM)
                # ema_x_int_new = exi*0.99 + xi*0.01 (in place over t_exi)
                nc.vector.scalar_tensor_tensor(t_exi[:], t_exi[:],
                                               ONE_MINUS_MOM, t_delta[:],
                                               Alu.mult, Alu.add)
                # out = x_int * s_scale
                nc.vector.tensor_scalar(t_outp[:], t_xi[:], ss[:], None,
                                        Alu.mult)

                def store(name, t):
                    nc.sync.dma_start(d_out[name][r0:r0 + P, c0:c0 + FD], t[:])

                store("out", t_outp)
                store("prev_x_int_new", t_xi)
                store("prev_switch_dir_new", t_eq)
                store("ema_osc_new", t_ema)
                store("frozen_new", t_fnew)
                store("frozen_x_int_new", t_fxi)
                store("ema_x_int_new", t_exi)

            tot = small.tile([P, 1], F32, tag="tot")
            nc.vector.tensor_reduce(tot[:], acc[:], axis=mybir.AxisListType.X,
                                    op=Alu.add)
            nc.sync.dma_start(d_osc[r0:r0 + P, :], tot[:])

    nc.compile()
    return nc


def _get_module():
    global _MODULE
    if _MODULE is None:
        _MODULE = _build_module()
    return _MODULE


def _host_s_scale(s):
    # Reproduce the reference forward value bit-exactly in fp32:
    # clipped = where(s > eps, s, eps); s_scale = (clipped - s*g) + s*g
    s = s.astype(np.float32)
    g = np.float32(1.0 / np.sqrt(7 * IN_DIM))
    clipped = np.where(s > np.float32(1e-5), s, np.float32(1e-5)).astype(np.float32)
    sg = (s * g).astype(np.float32)
    return ((clipped - sg) + sg).astype(np.float32)


def run_shards(inputs, trace=False):
    """Shard inputs, run the SPMD kernel on 8 cores, return (outputs, results)."""
    nc = _get_module()
    x = np.ascontiguousarray(np.asarray(inputs["x"], dtype=np.float32))
    s = np.asarray(inputs["s"], dtype=np.float32).reshape(OUT_DIM)
    ss_full = _host_s_scale(s).reshape(OUT_DIM, 1)

    sr_full = np.divide(np.float32(1.0), ss_full, dtype=np.float32)
    full = {
        "x": x,
        "s_recip": sr_full,
        "prev_x_int": np.ascontiguousarray(np.asarray(inputs["prev_x_int"], np.float32)),
        "prev_switch_dir": np.ascontiguousarray(np.asarray(inputs["prev_switch_dir"], np.float32)),
        "ema_oscillation": np.ascontiguousarray(np.asarray(inputs["ema_oscillation"], np.float32)),
        "frozen": np.ascontiguousarray(np.asarray(inputs["frozen"]).astype(np.uint8)),
        "frozen_x_int": np.ascontiguousarray(np.asarray(inputs["frozen_x_int"], np.float32)),
        "ema_x_int": np.ascontiguousarray(np.asarray(inputs["ema_x_int"], np.float32)),
        "s_scale": ss_full,
    }
    in_maps = []
    for c in range(N_CORES):
        sl = slice(c * ROWS, (c + 1) * ROWS)
        in_maps.append({k: np.ascontiguousarray(v[sl]) for k, v in full.items()})

    res = run_bass_kernel_spmd(nc, in_maps, list(range(N_CORES)), trace=trace)

    outs = {}
    for name in F32_OUT + ["frozen_new"]:
        outs[name] = np.concatenate([res.results[c][name] for c in range(N_CORES)],
                                    axis=0)
    osc = np.concatenate([res.results[c]["osc_rowsum"] for c in range(N_CORES)])
    osc_sum = np.float32(np.sum(osc.astype(np.float64)))
    return outs, osc_sum, res


def kernel(**inputs):
    outs, osc_sum, _ = run_shards(inputs, trace=bool(os.environ.get("KERNEL_TRACE")))
    return (
        outs["out"],
        outs["prev_x_int_new"],
        outs["prev_switch_dir_new"],
        outs["ema_osc_new"],
        np.asarray(osc_sum, dtype=np.float32),
        outs["frozen_new"].astype(bool),
        outs["frozen_x_int_new"],
        outs["ema_x_int_new"],
    )


# revision 15
# speedup vs baseline: 2.0471x; 2.0471x over previous
"""Trainium2 Bass kernel for LSQ weight quantizer with iterative freezing.

Pure elementwise over a [4096, 11008] weight tensor plus per-row scale.
Sharded row-wise across 8 NeuronCores (512 rows each); no communication.

Per-core layout: [512, 11008] -> 4 row-blocks of 128 partitions, free dim
tiled into chunks. All state updates are elementwise; oscillated_sum is a
full reduction finished on host (exact: summands are 0/1 ints).
"""

import os
from contextlib import ExitStack

import numpy as np

import concourse.bass as bass
import concourse.bacc as bacc
import concourse.mybir as mybir
import concourse.tile as tile
from concourse.bass_utils import run_bass_kernel_spmd

OUT_DIM, IN_DIM = 4096, 11008
N_CORES = 8
ROWS = OUT_DIM // N_CORES  # 512 rows per core
P = 128                    # SBUF partitions
RB = ROWS // P             # 4 row blocks per core
FD = 1376                  # free-dim tile size
CS = IN_DIM // FD          # 8 col steps per row block

F32 = mybir.dt.float32
U8 = mybir.dt.uint8
Alu = mybir.AluOpType
Act = mybir.ActivationFunctionType

MAGIC = float(np.float32(12582912.0))  # 1.5*2^23: (v+C)-C == round-half-even(v)
THD_NEG = -8.0
THD_POS = 7.0
MOM = float(np.float32(0.01))
ONE_MINUS_MOM = float(np.float32(0.99))
FREEZE_THR = float(np.float32(0.01))

F32_BIG = ["x", "prev_x_int", "prev_switch_dir", "ema_oscillation",
           "frozen_x_int", "ema_x_int"]
F32_OUT = ["out", "prev_x_int_new", "prev_switch_dir_new", "ema_osc_new",
           "frozen_x_int_new", "ema_x_int_new"]

_MODULE = None


def _build_module(n_rb=RB, repeat=1):
    nc = bacc.Bacc(None)
    d_in = {n: nc.declare_dram_parameter(n, [ROWS, IN_DIM], F32, isOutput=False)
            for n in F32_BIG}
    d_in["frozen"] = nc.declare_dram_parameter("frozen", [ROWS, IN_DIM], U8,
                                               isOutput=False)
    d_ss = nc.declare_dram_parameter("s_scale", [ROWS, 1], F32, isOutput=False)
    d_sr = nc.declare_dram_parameter("s_recip", [ROWS, 1], F32, isOutput=False)
    d_out = {n: nc.declare_dram_parameter(n, [ROWS, IN_DIM], F32, isOutput=True)
             for n in F32_OUT}
    d_out["frozen_new"] = nc.declare_dram_parameter("frozen_new", [ROWS, IN_DIM],
                                                    U8, isOutput=True)
    d_osc = nc.declare_dram_parameter("osc_rowsum", [ROWS, 1], F32, isOutput=True)

    with tile.TileContext(nc) as tc, ExitStack() as ctx:
        pin = ctx.enter_context(tc.tile_pool(name="pin", bufs=3))
        big = ctx.enter_context(tc.tile_pool(name="big", bufs=2))
        small = ctx.enter_context(tc.tile_pool(name="small", bufs=2))
        if repeat > 1:
            ctx.enter_context(tc.For_i(0, repeat, 1))

        for rb in range(n_rb):
            r0 = rb * P
            ss = small.tile([P, 1], F32, tag="ss")
            nc.sync.dma_start(ss[:], d_ss[r0:r0 + P, :])
            sr = small.tile([P, 1], F32, tag="sr")
            nc.sync.dma_start(sr[:], d_sr[r0:r0 + P, :])
            acc = small.tile([P, CS], F32, tag="acc")

            for cs in range(CS):
                c0 = cs * FD

                def load(name, dt=F32, tag=None):
                    t = pin.tile([P, FD], dt, tag=tag or name)
                    nc.sync.dma_start(t[:], d_in[name][r0:r0 + P, c0:c0 + FD])
                    return t

                t_x = load("x")
                t_pxi = load("prev_x_int")
                t_psd = load("prev_switch_dir")
                t_ema = load("ema_oscillation")
                t_frz = load("frozen", U8)
                t_fxi = load("frozen_x_int")
                t_exi = load("ema_x_int")

                t_xi = big.tile([P, FD], F32, tag="xi")
                t_sd = big.tile([P, FD], F32, tag="sd")
                t_delta = big.tile([P, FD], F32, tag="delta")
                t_eq = big.tile([P, FD], F32, tag="eq")
                t_prod = big.tile([P, FD], F32, tag="prod")
                t_rexi = big.tile([P, FD], F32, tag="rexi")
                t_outp = big.tile([P, FD], F32, tag="outp")
                t_mf = big.tile([P, FD], U8, tag="mf")
                t_fnew = big.tile([P, FD], U8, tag="fnew")

                # --- fake quant: x_int = clip(round(x/s), -8, 7); frozen mask ---
                # x/s lowered as x * RN(1/s), matching the XLA-neuron divide
                nc.vector.tensor_scalar(t_xi[:], t_x[:], sr[:], None, Alu.mult)
                nc.vector.tensor_scalar(t_xi[:], t_xi[:], MAGIC, MAGIC,
                                        Alu.add, Alu.subtract)
                nc.vector.tensor_scalar(t_xi[:], t_xi[:], THD_POS, THD_NEG,
                                        Alu.min, Alu.max)
                nc.vector.copy_predicated(t_xi[:], t_frz[:], t_fxi[:])

                # --- oscillation tracking ---
                nc.gpsimd.tensor_tensor(t_delta[:], t_pxi[:], t_xi[:],
                                        Alu.subtract)
                nc.scalar.sign(t_sd[:], t_delta[:])
                # eq*psd = (delta == 0) * prev_switch_dir
                nc.vector.scalar_tensor_tensor(t_eq[:], t_delta[:], 0.0,
                                               t_psd[:], Alu.is_equal, Alu.mult)
                # prod = psd * sd (before psd is dead)
                nc.vector.tensor_tensor(t_prod[:], t_psd[:], t_sd[:], Alu.mult)
                # psd_new = sd + eq*psd (in place over t_eq)
                nc.vector.tensor_tensor(t_eq[:], t_eq[:], t_sd[:], Alu.add)
                # oscillated = (prod == -1), with row-sum accumulated
                nc.vector.tensor_scalar(t_prod[:], t_prod[:], -1.0, None,
                                        Alu.is_equal, Alu.add,
                                        accum_out=acc[:, cs:cs + 1])
                # osc*0.01 on ACT (reuse t_sd)
                nc.scalar.mul(t_sd[:], t_prod[:], MOM)
                # ema_new = ema*0.99 + osc*0.01 (in place over t_ema)
                nc.vector.scalar_tensor_tensor(t_ema[:], t_ema[:],
                                               ONE_MINUS_MOM, t_sd[:],
                                               Alu.mult, Alu.add)
                # freeze_w = ema_new > 0.01 (u8 mask)
                nc.vector.tensor_scalar(t_mf[:], t_ema[:], FREEZE_THR, None,
                                        Alu.is_gt)
                # frozen_new = frozen | freeze_w
                nc.vector.tensor_tensor(t_fnew[:], t_frz[:], t_mf[:],
                                        Alu.bitwise_or)
                # round(ema_x_int)
                nc.vector.tensor_scalar(t_rexi[:], t_exi[:], MAGIC, MAGIC,
                                        Alu.add, Alu.subtract)
                # frozen_x_int_new = where(freeze_w, round(exi), fxi) in place
                nc.vector.copy_predicated(t_fxi[:], t_mf[:], t_rexi[:])
                # xi*0.01 on ACT (reuse t_delta)
                nc.scalar.mul(t_delta[:], t_xi[:], MOM)
                # ema_x_int_new = exi*0.99 + xi*0.01 (in place over t_exi)
                nc.vector.scalar_tensor_tensor(t_exi[:], t_exi[:],
                                               ONE_MINUS_MOM, t_delta[:],
                                               Alu.mult, Alu.add)
                # out = x_int * s_scale
                nc.vector.tensor_scalar(t_outp[:], t_xi[:], ss[:], None,
                                        Alu.mult)

                def store(name, t):
                    # stores issue from the ACT sequencer so the SP
                    # sequencer's DMA-trigger processing doesn't serialize
                    nc.scalar.dma_start(d_out[name][r0:r0 + P, c0:c0 + FD], t[:])

                store("out", t_outp)
                store("prev_x_int_new", t_xi)
                store("prev_switch_dir_new", t_eq)
                store("ema_osc_new", t_ema)
                store("frozen_new", t_fnew)
                store("frozen_x_int_new", t_fxi)
                store("ema_x_int_new", t_exi)

            tot = small.tile([P, 1], F32, tag="tot")
            nc.vector.tensor_reduce(tot[:], acc[:], axis=mybir.AxisListType.X,
                                    op=Alu.add)
            nc.scalar.dma_start(d_osc[r0:r0 + P, :], tot[:])

    nc.compile()
    return nc


def _get_module():
    global _MODULE
    if _MODULE is None:
        _MODULE = _build_module()
    return _MODULE


def _host_s_scale(s):
    # Reproduce the reference forward value bit-exactly in fp32:
    # clipped = where(s > eps, s, eps); s_scale = (clipped - s*g) + s*g
    s = s.astype(np.float32)
    g = np.float32(1.0 / np.sqrt(7 * IN_DIM))
    clipped = np.where(s > np.float32(1e-5), s, np.float32(1e-5)).astype(np.float32)
    sg = (s * g).astype(np.float32)
    return ((clipped - sg) + sg).astype(np.float32)


def run_shards(inputs, trace=False):
    """Shard inputs, run the SPMD kernel on 8 cores, return (outputs, results)."""
    nc = _get_module()
    x = np.ascontiguousarray(np.asarray(inputs["x"], dtype=np.float32))
    s = np.asarray(inputs["s"], dtype=np.float32).reshape(OUT_DIM)
    ss_full = _host_s_scale(s).reshape(OUT_DIM, 1)

    sr_full = np.divide(np.float32(1.0), ss_full, dtype=np.float32)
    full = {
        "x": x,
        "s_recip": sr_full,
        "prev_x_int": np.ascontiguousarray(np.asarray(inputs["prev_x_int"], np.float32)),
        "prev_switch_dir": np.ascontiguousarray(np.asarray(inputs["prev_switch_dir"], np.float32)),
        "ema_oscillation": np.ascontiguousarray(np.asarray(inputs["ema_oscillation"], np.float32)),
        "frozen": np.ascontiguousarray(np.asarray(inputs["frozen"]).astype(np.uint8)),
        "frozen_x_int": np.ascontiguousarray(np.asarray(inputs["frozen_x_int"], np.float32)),
        "ema_x_int": np.ascontiguousarray(np.asarray(inputs["ema_x_int"], np.float32)),
        "s_scale": ss_full,
    }
    in_maps = []
    for c in range(N_CORES):
        sl = slice(c * ROWS, (c + 1) * ROWS)
        in_maps.append({k: np.ascontiguousarray(v[sl]) for k, v in full.items()})

    res = run_bass_kernel_spmd(nc, in_maps, list(range(N_CORES)), trace=trace)

    outs = {}
    for name in F32_OUT + ["frozen_new"]:
        outs[name] = np.concatenate([res.results[c][name] for c in range(N_CORES)],
                                    axis=0)
    osc = np.concatenate([res.results[c]["osc_rowsum"] for c in range(N_CORES)])
    osc_sum = np.float32(np.sum(osc.astype(np.float64)))
    return outs, osc_sum, res


def kernel(**inputs):
    outs, osc_sum, _ = run_shards(inputs, trace=bool(os.environ.get("KERNEL_TRACE")))
    return (
        outs["out"],
        outs["prev_x_int_new"],
        outs["prev_switch_dir_new"],
        outs["ema_osc_new"],
        np.asarray(osc_sum, dtype=np.float32),
        outs["frozen_new"].astype(bool),
        outs["frozen_x_int_new"],
        outs["ema_x_int_new"],
    )


# revision 16
# speedup vs baseline: 2.3179x; 1.1323x over previous
"""Trainium2 Bass kernel for LSQ weight quantizer with iterative freezing.

Pure elementwise over a [4096, 11008] weight tensor plus per-row scale.
Sharded row-wise across 8 NeuronCores (512 rows each); no communication.

Per-core layout: [512, 11008] -> 4 row-blocks of 128 partitions, free dim
tiled into chunks. All state updates are elementwise; oscillated_sum is a
full reduction finished on host (exact: summands are 0/1 ints).
"""

import os
from contextlib import ExitStack

import numpy as np

import concourse.bass as bass
import concourse.bacc as bacc
import concourse.mybir as mybir
import concourse.tile as tile
from concourse.bass_utils import run_bass_kernel_spmd

OUT_DIM, IN_DIM = 4096, 11008
N_CORES = 8
ROWS = OUT_DIM // N_CORES  # 512 rows per core
P = 128                    # SBUF partitions
RB = ROWS // P             # 4 row blocks per core
FD = 1376                  # free-dim tile size
CS = IN_DIM // FD          # 8 col steps per row block

F32 = mybir.dt.float32
U8 = mybir.dt.uint8
Alu = mybir.AluOpType
Act = mybir.ActivationFunctionType

MAGIC = float(np.float32(12582912.0))  # 1.5*2^23: (v+C)-C == round-half-even(v)
THD_NEG = -8.0
THD_POS = 7.0
MOM = float(np.float32(0.01))
ONE_MINUS_MOM = float(np.float32(0.99))
FREEZE_THR = float(np.float32(0.01))

F32_BIG = ["x", "prev_x_int", "prev_switch_dir", "ema_oscillation",
           "frozen_x_int", "ema_x_int"]
F32_OUT = ["out", "prev_x_int_new", "prev_switch_dir_new", "ema_osc_new",
           "frozen_x_int_new", "ema_x_int_new"]

_MODULE = None


def _build_module(n_rb=RB, repeat=1):
    nc = bacc.Bacc(None)
    d_in = {n: nc.declare_dram_parameter(n, [ROWS, IN_DIM], F32, isOutput=False)
            for n in F32_BIG}
    d_in["frozen"] = nc.declare_dram_parameter("frozen", [ROWS, IN_DIM], U8,
                                               isOutput=False)
    d_ss = nc.declare_dram_parameter("s_scale", [ROWS, 1], F32, isOutput=False)
    d_sr = nc.declare_dram_parameter("s_recip", [ROWS, 1], F32, isOutput=False)
    d_out = {n: nc.declare_dram_parameter(n, [ROWS, IN_DIM], F32, isOutput=True)
             for n in F32_OUT}
    d_out["frozen_new"] = nc.declare_dram_parameter("frozen_new", [ROWS, IN_DIM],
                                                    U8, isOutput=True)
    d_osc = nc.declare_dram_parameter("osc_rowsum", [ROWS, 1], F32, isOutput=True)

    with tile.TileContext(nc) as tc, ExitStack() as ctx:
        pin = ctx.enter_context(tc.tile_pool(name="pin", bufs=3))
        big = ctx.enter_context(tc.tile_pool(name="big", bufs=2))
        small = ctx.enter_context(tc.tile_pool(name="small", bufs=2))
        if repeat > 1:
            ctx.enter_context(tc.For_i(0, repeat, 1))

        for rb in range(n_rb):
            r0 = rb * P
            ss = small.tile([P, 1], F32, tag="ss")
            nc.sync.dma_start(ss[:], d_ss[r0:r0 + P, :])
            sr = small.tile([P, 1], F32, tag="sr")
            nc.sync.dma_start(sr[:], d_sr[r0:r0 + P, :])
            acc = small.tile([P, CS], F32, tag="acc")

            for cs in range(CS):
                c0 = cs * FD

                _ld = [0]

                def load(name, dt=F32, tag=None):
                    t = pin.tile([P, FD], dt, tag=tag or name)
                    eng = nc.sync if _ld[0] % 2 == 0 else nc.scalar
                    _ld[0] += 1
                    eng.dma_start(t[:], d_in[name][r0:r0 + P, c0:c0 + FD])
                    return t

                t_x = load("x")
                t_pxi = load("prev_x_int")
                t_psd = load("prev_switch_dir")
                t_ema = load("ema_oscillation")
                t_frz = load("frozen", U8)
                t_fxi = load("frozen_x_int")
                t_exi = load("ema_x_int")

                t_xi = big.tile([P, FD], F32, tag="xi")
                t_sd = big.tile([P, FD], F32, tag="sd")
                t_delta = big.tile([P, FD], F32, tag="delta")
                t_eq = big.tile([P, FD], F32, tag="eq")
                t_prod = big.tile([P, FD], F32, tag="prod")
                t_rexi = big.tile([P, FD], F32, tag="rexi")
                t_outp = big.tile([P, FD], F32, tag="outp")
                t_mf = big.tile([P, FD], U8, tag="mf")
                t_fnew = big.tile([P, FD], U8, tag="fnew")

                # --- fake quant: x_int = clip(round(x/s), -8, 7); frozen mask ---
                # x/s lowered as x * RN(1/s), matching the XLA-neuron divide
                nc.vector.tensor_scalar(t_xi[:], t_x[:], sr[:], None, Alu.mult)
                nc.vector.tensor_scalar(t_xi[:], t_xi[:], MAGIC, MAGIC,
                                        Alu.add, Alu.subtract)
                nc.vector.tensor_scalar(t_xi[:], t_xi[:], THD_POS, THD_NEG,
                                        Alu.min, Alu.max)
                nc.vector.copy_predicated(t_xi[:], t_frz[:], t_fxi[:])

                # --- oscillation tracking ---
                nc.gpsimd.tensor_tensor(t_delta[:], t_pxi[:], t_xi[:],
                                        Alu.subtract)
                nc.scalar.sign(t_sd[:], t_delta[:])
                # eq*psd = (delta == 0) * prev_switch_dir
                nc.vector.scalar_tensor_tensor(t_eq[:], t_delta[:], 0.0,
                                               t_psd[:], Alu.is_equal, Alu.mult)
                # prod = psd * sd (before psd is dead)
                nc.vector.tensor_tensor(t_prod[:], t_psd[:], t_sd[:], Alu.mult)
                # psd_new = sd + eq*psd (in place over t_eq)
                nc.vector.tensor_tensor(t_eq[:], t_eq[:], t_sd[:], Alu.add)
                # oscillated = (prod == -1), with row-sum accumulated
                nc.vector.tensor_scalar(t_prod[:], t_prod[:], -1.0, None,
                                        Alu.is_equal, Alu.add,
                                        accum_out=acc[:, cs:cs + 1])
                # osc*0.01 on ACT (reuse t_sd)
                nc.scalar.mul(t_sd[:], t_prod[:], MOM)
                # ema_new = ema*0.99 + osc*0.01 (in place over t_ema)
                nc.vector.scalar_tensor_tensor(t_ema[:], t_ema[:],
                                               ONE_MINUS_MOM, t_sd[:],
                                               Alu.mult, Alu.add)
                # freeze_w = ema_new > 0.01 (u8 mask)
                nc.vector.tensor_scalar(t_mf[:], t_ema[:], FREEZE_THR, None,
                                        Alu.is_gt)
                # frozen_new = frozen | freeze_w
                nc.vector.tensor_tensor(t_fnew[:], t_frz[:], t_mf[:],
                                        Alu.bitwise_or)
                # round(ema_x_int)
                nc.vector.tensor_scalar(t_rexi[:], t_exi[:], MAGIC, MAGIC,
                                        Alu.add, Alu.subtract)
                # frozen_x_int_new = where(freeze_w, round(exi), fxi) in place
                nc.vector.copy_predicated(t_fxi[:], t_mf[:], t_rexi[:])
                # xi*0.01 on ACT (reuse t_delta)
                nc.scalar.mul(t_delta[:], t_xi[:], MOM)
                # ema_x_int_new = exi*0.99 + xi*0.01 (in place over t_exi)
                nc.vector.scalar_tensor_tensor(t_exi[:], t_exi[:],
                                               ONE_MINUS_MOM, t_delta[:],
                                               Alu.mult, Alu.add)
                # out = x_int * s_scale
                nc.vector.tensor_scalar(t_outp[:], t_xi[:], ss[:], None,
                                        Alu.mult)

                _st = [0]

                def store(name, t):
                    # alternate store triggers between the two HWDGE engines
                    eng = nc.scalar if _st[0] % 2 == 0 else nc.sync
                    _st[0] += 1
                    eng.dma_start(d_out[name][r0:r0 + P, c0:c0 + FD], t[:])

                store("out", t_outp)
                store("prev_x_int_new", t_xi)
                store("prev_switch_dir_new", t_eq)
                store("ema_osc_new", t_ema)
                store("frozen_new", t_fnew)
                store("frozen_x_int_new", t_fxi)
                store("ema_x_int_new", t_exi)

            tot = small.tile([P, 1], F32, tag="tot")
            nc.vector.tensor_reduce(tot[:], acc[:], axis=mybir.AxisListType.X,
                                    op=Alu.add)
            nc.scalar.dma_start(d_osc[r0:r0 + P, :], tot[:])

    nc.compile()
    return nc


def _get_module():
    global _MODULE
    if _MODULE is None:
        _MODULE = _build_module()
    return _MODULE


def _host_s_scale(s):
    # Reproduce the reference forward value bit-exactly in fp32:
    # clipped = where(s > eps, s, eps); s_scale = (clipped - s*g) + s*g
    s = s.astype(np.float32)
    g = np.float32(1.0 / np.sqrt(7 * IN_DIM))
    clipped = np.where(s > np.float32(1e-5), s, np.float32(1e-5)).astype(np.float32)
    sg = (s * g).astype(np.float32)
    return ((clipped - sg) + sg).astype(np.float32)


def run_shards(inputs, trace=False):
    """Shard inputs, run the SPMD kernel on 8 cores, return (outputs, results)."""
    nc = _get_module()
    x = np.ascontiguousarray(np.asarray(inputs["x"], dtype=np.float32))
    s = np.asarray(inputs["s"], dtype=np.float32).reshape(OUT_DIM)
    ss_full = _host_s_scale(s).reshape(OUT_DIM, 1)

    sr_full = np.divide(np.float32(1.0), ss_full, dtype=np.float32)
    full = {
        "x": x,
        "s_recip": sr_full,
        "prev_x_int": np.ascontiguousarray(np.asarray(inputs["prev_x_int"], np.float32)),
        "prev_switch_dir": np.ascontiguousarray(np.asarray(inputs["prev_switch_dir"], np.float32)),
        "ema_oscillation": np.ascontiguousarray(np.asarray(inputs["ema_oscillation"], np.float32)),
        "frozen": np.ascontiguousarray(np.asarray(inputs["frozen"]).astype(np.uint8)),
        "frozen_x_int": np.ascontiguousarray(np.asarray(inputs["frozen_x_int"], np.float32)),
        "ema_x_int": np.ascontiguousarray(np.asarray(inputs["ema_x_int"], np.float32)),
        "s_scale": ss_full,
    }
    in_maps = []
    for c in range(N_CORES):
        sl = slice(c * ROWS, (c + 1) * ROWS)
        in_maps.append({k: np.ascontiguousarray(v[sl]) for k, v in full.items()})

    res = run_bass_kernel_spmd(nc, in_maps, list(range(N_CORES)), trace=trace)

    outs = {}
    for name in F32_OUT + ["frozen_new"]:
        outs[name] = np.concatenate([res.results[c][name] for c in range(N_CORES)],
                                    axis=0)
    osc = np.concatenate([res.results[c]["osc_rowsum"] for c in range(N_CORES)])
    osc_sum = np.float32(np.sum(osc.astype(np.float64)))
    return outs, osc_sum, res


def kernel(**inputs):
    outs, osc_sum, _ = run_shards(inputs, trace=bool(os.environ.get("KERNEL_TRACE")))
    return (
        outs["out"],
        outs["prev_x_int_new"],
        outs["prev_switch_dir_new"],
        outs["ema_osc_new"],
        np.asarray(osc_sum, dtype=np.float32),
        outs["frozen_new"].astype(bool),
        outs["frozen_x_int_new"],
        outs["ema_x_int_new"],
    )
